# revision 9
# baseline (speedup 1.0000x reference)
"""Trainium2 Bass kernel for nn_PiNet (degree-3 polynomial network).

out = b + x@W1^T + kron2(x)@W2^T + kron3(x)@W3^T
with B=256, IN=64, OUT=512  (W3: [512, 262144], ~69 GFLOP dominant term).

Strategy (8 NeuronCores, SPMD):
  - Never materialize kron3. Using z3[b, i*4096+jk] = x[b,i]*z2[b,jk]:
        out3 = sum_i diag(x[:,i]) @ (Z2 @ W3_i^T)
    where W3_i = W3[:, i*4096:(i+1)*4096]. The diag-scale is a cheap
    per-partition tensor_scalar on the [128,512] matmul result.
  - Shard W3 column-wise over the kron3 axis: core c owns i in [8c, 8c+8),
    i.e. a contiguous [512, 32768] block of W3. Also shard W2's contraction
    (512 rows of Z2T each) and give every core W1/8 and b/8, so the sum of
    the 8 partial outputs (host-side all-reduce) is exactly the full output.
  - All matmul operands in bf16 (fp32 PSUM accumulation); measured overall
    relative error ~1.6e-3 vs the fp32 reference.
  - Host pre-tiles weights into DMA-linear layouts; per-core W3 shard is
    streamed as 8 x 4MB double-buffered tiles while the PE consumes them.
"""

import sys

for _p in ("/opt/trn_rl_repo",):
    if _p not in sys.path:
        sys.path.append(_p)

import numpy as np
import ml_dtypes

B = 256
IN = 64
OUT = 512
NCORES = 8
IPC = IN // NCORES          # 8 i-values per core
JK = IN * IN                # 4096
MCH = JK // 128             # 32 z2t chunks of 128
BCH = B // 128              # 2 batch chunks

BF16 = ml_dtypes.bfloat16

_NC = None  # cached compiled Bass module

TRACE = False
LAST_EXEC_NS = None
LAST_RESULTS = None


def _build_nc():
    import concourse.mybir as mybir
    import concourse.tile as tile
    from concourse import bacc

    bf = mybir.dt.bfloat16
    f32 = mybir.dt.float32

    nc = bacc.Bacc(None, target_bir_lowering=False, debug=False)

    z2t_d = nc.dram_tensor("z2t", [128, MCH, B], bf, kind="ExternalInput")
    w3t_d = nc.dram_tensor("w3t", [IPC, 128, MCH, OUT], bf, kind="ExternalInput")
    z2w2_d = nc.dram_tensor("z2w2", [128, 4, B], bf, kind="ExternalInput")
    w2t_d = nc.dram_tensor("w2t", [128, 4, OUT], bf, kind="ExternalInput")
    xt_d = nc.dram_tensor("xt", [IN, B], bf, kind="ExternalInput")
    w1t_d = nc.dram_tensor("w1t", [IN, OUT], bf, kind="ExternalInput")
    xcols_d = nc.dram_tensor("xcols", [128, BCH * IPC], f32, kind="ExternalInput")
    biast_d = nc.dram_tensor("biast", [128, BCH, OUT], f32, kind="ExternalInput")
    out_d = nc.dram_tensor("out", [BCH, 128, OUT], f32, kind="ExternalOutput")

    MULT = mybir.AluOpType.mult
    ADD = mybir.AluOpType.add

    # split the z2t stationary and the FIRST W3 stream into sub-tiles so the
    # first matmuls only wait for the first ~1.5MB of DMA, not ~6.3MB.
    # (Splitting every W3 tile regressed steady-state matmul throughput —
    # more concurrent DMA streams slow the PE slices — so only i=0 is split.)
    ZSPLIT = 4                  # z2t in 4 pieces of 8 m-chunks
    ZM = MCH // ZSPLIT
    WSPLIT = 4                  # the first 4MB W3 i-tile in 4 pieces of 1MB
    WM = MCH // WSPLIT

    with tile.TileContext(nc) as tc:
        with (
            tc.tile_pool(name="consts", bufs=1) as cpool,
            tc.tile_pool(name="w3", bufs=2) as w3pool,
            tc.tile_pool(name="w3s", bufs=WSPLIT) as w3spool,
            tc.tile_pool(name="acc", bufs=1) as apool,
            tc.tile_pool(name="psum", bufs=4, space="PSUM") as ppool,
        ):
            z2t = cpool.tile([128, MCH, B], bf)
            z2w2 = cpool.tile([128, 4, B], bf)
            w2t = cpool.tile([128, 4, OUT], bf)
            xt = cpool.tile([IN, B], bf)
            w1t = cpool.tile([IN, OUT], bf)
            xcols = cpool.tile([128, BCH * IPC], f32)
            acc = apool.tile([128, BCH, OUT], f32)

            # z2t gates every W3 matmul: lead the SP ring with it (that ring
            # starts ~2.5us earlier than ACT). Small consts go on the ACT ring.
            nc.sync.dma_start(z2t[:, :, :], z2t_d[:, :, :])
            nc.scalar.dma_start(z2w2[:, :, :], z2w2_d[:, :, :])
            nc.scalar.dma_start(w2t[:, :, :], w2t_d[:, :, :])
            nc.scalar.dma_start(xt[:, :], xt_d[:, :])
            nc.scalar.dma_start(w1t[:, :], w1t_d[:, :])
            nc.scalar.dma_start(xcols[:, :], xcols_d[:, :])
            nc.scalar.dma_start(acc[:, :, :], biast_d[:, :, :])

            # W2 partial (4 z2t chunks of this core's slice) + W1/8 term,
            # run FIRST so the PE has work while the first W3 tile streams
            for bc in range(BCH):
                ps2 = ppool.tile([128, OUT], f32, tag="ps", name=f"ps2_{bc}")
                for m in range(4):
                    nc.tensor.matmul(
                        ps2[:, :],
                        z2w2[:, m, 128 * bc : 128 * (bc + 1)],
                        w2t[:, m, :],
                        start=(m == 0),
                        stop=False,
                    )
                nc.tensor.matmul(
                    ps2[:, :],
                    xt[:, 128 * bc : 128 * (bc + 1)],
                    w1t[:, :],
                    start=False,
                    stop=True,
                )
                nc.vector.scalar_tensor_tensor(
                    acc[:, bc, :], ps2[:, :], 1.0, acc[:, bc, :], MULT, ADD
                )

            for i in range(IPC):
                if i == 0:
                    w3p = [
                        w3spool.tile([128, WM, OUT], bf, tag="w3s", name=f"w3sb0_{w}")
                        for w in range(WSPLIT)
                    ]
                    for w in range(WSPLIT):
                        nc.sync.dma_start(
                            w3p[w][:, :, :], w3t_d[0, :, WM * w : WM * (w + 1), :]
                        )
                    rhs = lambda m: w3p[m // WM][:, m % WM, :]
                else:
                    w3sb = w3pool.tile([128, MCH, OUT], bf, tag="w3", name=f"w3sb_{i}")
                    # odd i on the ACT ring, even on SP: the two rings drain in
                    # parallel so neither prefetch starves behind the z2t load
                    eng = nc.scalar if i % 2 == 1 else nc.sync
                    eng.dma_start(w3sb[:, :, :], w3t_d[i, :, :, :])
                    rhs = lambda m: w3sb[:, m, :]
                ps = [ppool.tile([128, OUT], f32, tag="ps", name=f"ps_{i}_{bc}") for bc in range(BCH)]
                for m in range(MCH):
                    for bc in range(BCH):
                        nc.tensor.matmul(
                            ps[bc][:, :],
                            z2t[:, m, 128 * bc : 128 * (bc + 1)],
                            rhs(m),
                            start=(m == 0),
                            stop=(m == MCH - 1),
                        )
                for bc in range(BCH):
                    # acc += x[:, 8c+i] * ps   (fused multiply-add on DVE)
                    nc.vector.scalar_tensor_tensor(
                        acc[:, bc, :],
                        ps[bc][:, :],
                        xcols[:, bc * IPC + i : bc * IPC + i + 1],
                        acc[:, bc, :],
                        MULT,
                        ADD,
                    )

            for bc in range(BCH):
                nc.sync.dma_start(out_d[bc, :, :], acc[:, bc, :])

    nc.compile()
    return nc


def _get_nc():
    global _NC
    if _NC is None:
        _NC = _build_nc()
    return _NC


def _prep_inputs(x, W1, W2, W3, b):
    """Host-side shard + retile. Returns list of 8 in_maps."""
    x = np.ascontiguousarray(x, dtype=np.float32)
    W1 = np.ascontiguousarray(W1, dtype=np.float32)
    W2 = np.ascontiguousarray(W2, dtype=np.float32)
    W3 = np.ascontiguousarray(W3, dtype=np.float32)
    b = np.ascontiguousarray(b, dtype=np.float32)

    # z2[b, j*64+k] = x[b,j]*x[b,k]; products in fp32, rounded once to bf16
    z2 = (x[:, :, None] * x[:, None, :]).reshape(B, JK)
    z2t = np.ascontiguousarray(z2.T)                        # [4096, 256] f32
    z2t_tiled = np.ascontiguousarray(
        z2t.reshape(MCH, 128, B).transpose(1, 0, 2)
    ).astype(BF16)                                          # [128, 32, 256]

    xt = np.ascontiguousarray(x.T).astype(BF16)             # [64, 256]
    w1t = np.ascontiguousarray(W1.T / 8).astype(BF16)       # [64, 512]

    # W3 tiled: [c, i, p, m, o] with element W3[o, (8c+i)*4096 + m*128 + p]
    w3_tiled = np.ascontiguousarray(
        W3.astype(BF16).reshape(OUT, NCORES, IPC, MCH, 128).transpose(1, 2, 4, 3, 0)
    )                                                       # [8, 8, 128, 32, 512]

    w2T = np.ascontiguousarray(W2.T)                        # [4096, 512] f32
    biast = np.ascontiguousarray(
        np.broadcast_to((b / 8)[None, None, :], (128, BCH, OUT))
    ).astype(np.float32)

    in_maps = []
    for c in range(NCORES):
        z2w2_c = np.ascontiguousarray(
            z2t[512 * c : 512 * (c + 1)].reshape(4, 128, B).transpose(1, 0, 2)
        ).astype(BF16)                                      # [128, 4, 256]
        w2t_c = np.ascontiguousarray(
            w2T[512 * c : 512 * (c + 1)].astype(BF16).reshape(4, 128, OUT).transpose(1, 0, 2)
        )                                                   # [128, 4, 512]
        xcols_c = np.ascontiguousarray(
            x[:, IPC * c : IPC * (c + 1)].reshape(BCH, 128, IPC).transpose(1, 0, 2)
        ).reshape(128, BCH * IPC)                           # [128, 16] f32
        in_maps.append(
            {
                "z2t": z2t_tiled,
                "w3t": w3_tiled[c],
                "z2w2": z2w2_c,
                "w2t": w2t_c,
                "xt": xt,
                "w1t": w1t,
                "xcols": xcols_c,
                "biast": biast,
            }
        )
    return in_maps


def kernel(x, W1, W2, W3, b):
    from concourse.bass_utils import run_bass_kernel_spmd

    global LAST_EXEC_NS, LAST_RESULTS
    nc = _get_nc()
    in_maps = _prep_inputs(x, W1, W2, W3, b)
    res = run_bass_kernel_spmd(
        nc, in_maps, core_ids=list(range(NCORES)), trace=TRACE
    )
    LAST_EXEC_NS = res.exec_time_ns
    LAST_RESULTS = res
    total = np.zeros((BCH, 128, OUT), dtype=np.float64)
    for c in range(NCORES):
        total += res.results[c]["out"]
    return total.reshape(B, OUT).astype(np.float32)
